# revision 45
# baseline (speedup 1.0000x reference)
"""Grouped-Query Attention (B=2, S=2048, DIM=2048, 32 Q heads / 8 KV heads,
HD=64, RoPE, causal) on 8 Trainium2 NeuronCores.

Sharding: hybrid batch x tensor parallel. Core c handles batch b=c//4 and
head-group cp=c%4 (2 KV heads, 8 Q heads). Wq/Wk/Wv are column-sharded.
Each core outputs rows [c*256,(c+1)*256) of BOTH batches: an 8-core bf16
AllToAll of the per-head context makes every (src,dst) shard meaningful and
each core then multiplies its fully-gathered 2048-feature context slice by
the full Wo locally — no reduction.

All matmuls use bf16 inputs with fp32 PSUM accumulation. Activations stay
transposed [feature, token] so matmul contractions have the contraction dim
on partitions:
  qT = Wq^T x^T (RoPE on partition dim), kT likewise (each kv head
  duplicated at partition offsets 0/64 so the score matmuls of a q-head
  pair land in disjoint PE row groups and run concurrently),
  scoresT[kv, row] = kT^T qT, two kv tiles paired per 2-bank PSUM tile so
  one Exp activation covers 1024 columns,
  probsT = exp(scale*scoresT) in bf16 (no max subtraction: |scores*scale|
  < ~8 for this input distribution; softmax is shift-invariant),
  v is projected feature-major then PE-transposed to token-major with a
  ones column -> partition 64 of the ctx accumulator is the softmax
  denominator for free; it is broadcast with a rank-1 matmul,
  reciprocal'd (fast approx) on 64 lanes, and multiplied in.
Every PSUM tile holds exactly one matmul accumulation group — interleaved
start/stop groups inside one bank clobber each other on hardware.
"""

import numpy as np
from contextlib import ExitStack

import sys

if "/opt/trn_rl_repo" not in sys.path:
    sys.path.insert(0, "/opt/trn_rl_repo")

import ml_dtypes
import concourse.bass as bass
import concourse.bacc as bacc
import concourse.tile as tile
from concourse import mybir
from concourse.bass_utils import run_bass_kernel_spmd
from concourse.masks import make_identity

F32 = mybir.dt.float32
BF16 = mybir.dt.bfloat16
AF = mybir.ActivationFunctionType
NPBF = ml_dtypes.bfloat16

B, S, DIM = 2, 2048, 2048
QH, KVH, HD = 32, 8, 64
SCALE = HD ** -0.5

NCORES = 8
A2A_GROUP = [list(range(NCORES))]
QHL = 8            # q heads per core
KVHL = 2           # kv heads per core
QCOLS = QHL * HD   # 512
KCOLS = KVHL * HD  # 128
TOKC = 512         # token chunk (matmul N / PSUM bank width in fp32)
NTOK = S // TOKC   # 4
KT = DIM // 128    # 16 contraction tiles for the projections
OUT_ROWS = S // NCORES  # 256 output rows per core per batch


def _build_nc():
    nc = bacc.Bacc(None, num_devices=NCORES)

    xq = nc.declare_dram_parameter("xq", [DIM, S], BF16, isOutput=False)
    xk = nc.declare_dram_parameter("xk", [DIM, S], BF16, isOutput=False)
    xv = nc.declare_dram_parameter("xv", [DIM, S], BF16, isOutput=False)
    wq = nc.declare_dram_parameter("wq", [DIM, QCOLS], BF16, isOutput=False)
    wk = nc.declare_dram_parameter("wk", [DIM, KCOLS], BF16, isOutput=False)
    wv = nc.declare_dram_parameter("wv", [DIM, KCOLS], BF16, isOutput=False)
    wo = nc.declare_dram_parameter("wo", [DIM, DIM], BF16, isOutput=False)
    cosT = nc.declare_dram_parameter("cosT", [128, S], BF16, isOutput=False)
    sinT = nc.declare_dram_parameter("sinT", [128, S], BF16, isOutput=False)
    # mask[p, j, w, r] = 1.0 if 128*j + p <= r else 0.0 (causal mask for the
    # 4 diagonal kv tiles of each 512-token row chunk; duplicated along w so
    # one multiply covers both heads of a packed score tile)
    msk = nc.declare_dram_parameter("msk", [128, 4, 2, TOKC], BF16, isOutput=False)
    out_ext = nc.declare_dram_parameter("out", [B, OUT_ROWS, DIM], F32, isOutput=True)

    # AllToAll buffers: [dest/src core, local feat tile, 128, 256 rows]
    a2a_in = nc.dram_tensor("a2a_in", [NCORES, 4, 128, OUT_ROWS], BF16)
    a2a_out = nc.dram_tensor("a2a_out", [NCORES, 4, 128, OUT_ROWS], BF16)

    with tile.TileContext(nc) as tc, ExitStack() as ctx:
        const = ctx.enter_context(tc.tile_pool(name="const", bufs=1))
        wpool = ctx.enter_context(tc.tile_pool(name="wpool", bufs=1))
        qkv = ctx.enter_context(tc.tile_pool(name="qkv", bufs=1))
        qtp = ctx.enter_context(tc.tile_pool(name="qtp", bufs=2))
        xstream = ctx.enter_context(tc.tile_pool(name="xstream", bufs=3))
        probs = ctx.enter_context(tc.tile_pool(name="probs", bufs=3))
        ropet = ctx.enter_context(tc.tile_pool(name="ropet", bufs=2))
        denp = ctx.enter_context(tc.tile_pool(name="denp", bufs=4))
        ctxp = ctx.enter_context(tc.tile_pool(name="ctxp", bufs=2))
        ctxf = ctx.enter_context(tc.tile_pool(name="ctxf", bufs=1))
        orow_p = ctx.enter_context(tc.tile_pool(name="orow", bufs=2))
        ps_a = ctx.enter_context(tc.tile_pool(name="ps_a", bufs=2, space="PSUM"))
        ps_s = ctx.enter_context(tc.tile_pool(name="ps_s", bufs=2, space="PSUM"))
        ps_c = ctx.enter_context(tc.tile_pool(name="ps_c", bufs=2, space="PSUM"))

        # ---- constants / weights resident in SBUF ----
        ones1 = const.tile([1, 64], BF16, tag="ones1")
        nc.vector.memset(ones1, 1.0)
        # identity duplicated in both partition halves for the v transposes
        ident = const.tile([128, 64], BF16, tag="ident")
        make_identity(nc, ident[0:64, :])
        make_identity(nc, ident[64:128, :])

        # wq + chunk-0 x tiles gate the first matmul — load them first
        wq_sb = wpool.tile([128, KT, QCOLS], BF16, tag="wq")
        for kt in range(KT):
            nc.sync.dma_start(out=wq_sb[:, kt, :],
                              in_=wq[kt * 128:(kt + 1) * 128, :])
        wk_sb = wpool.tile([128, KT, KCOLS], BF16, tag="wk")
        nc.sync.dma_start(out=wk_sb, in_=wk.rearrange("(kt p) c -> p kt c", p=128))
        wv_sb = wpool.tile([128, KT, KCOLS], BF16, tag="wv")
        nc.sync.dma_start(out=wv_sb, in_=wv.rearrange("(kt p) c -> p kt c", p=128))

        cos_sb = const.tile([128, S], BF16, tag="cos")
        sin_sb = const.tile([128, S], BF16, tag="sin")
        for R in range(NTOK):
            sl = slice(R * TOKC, (R + 1) * TOKC)
            nc.sync.dma_start(out=cos_sb[:, sl], in_=cosT[:, sl])
            nc.sync.dma_start(out=sin_sb[:, sl], in_=sinT[:, sl])
        msk_sb = const.tile([128, 4, 2, TOKC], BF16, tag="msk")
        nc.sync.dma_start(out=msk_sb, in_=msk[:, :, :, :])

        # ---- persistent activations ----
        # kT_A = natural rope layout [kv0 @ 0-63, kv1 @ 64-127];
        # kT_B = swapped [kv1 @ 0-63, kv0 @ 64-127].  Head pairs are chosen
        # so each score matmul pair reads one of these directly.
        kT_A = qkv.tile([128, S], BF16, tag="ktA", name="ktA")
        kT_B = qkv.tile([128, S], BF16, tag="ktB", name="ktB")
        # v token-major with a ones column: [tok, kv_tile_idx, kv_head, 65]
        v_sb = qkv.tile([128, S // 128, KVHL, HD + 1], BF16, tag="v")
        nc.vector.memset(v_sb[:, :, :, HD:HD + 1], 1.0)

        xq_r = xq.rearrange("(k2 dt p) c -> p k2 dt c", dt=2, p=128)
        xk_r = xk.rearrange("(k2 dt p) c -> p k2 dt c", dt=2, p=128)
        xv_r = xv.rearrange("(k2 dt p) c -> p k2 dt c", dt=2, p=128)

        def rope_evict(ps, dst, cos_sl, sin_sl):
            """ps: [128, TOKC] fp32 PSUM with fresh projection; dst: bf16
            SBUF tile/slice. dst = ps*cos + rotate_half(ps)*sin.  The psum
            is first cast to bf16 on the Scalar engine so all DVE ops run
            in 2x/4x 16-bit modes."""
            raw = ropet.tile([128, TOKC], BF16, tag="rope_raw")
            nc.scalar.activation(raw, ps, AF.Copy)
            rot = ropet.tile([128, TOKC], BF16, tag="rot")
            for h0 in (0, 64):
                nc.vector.tensor_copy(rot[h0:h0 + 32, :], raw[h0 + 32:h0 + 64, :])
                nc.vector.tensor_copy(rot[h0 + 32:h0 + 64, :], raw[h0:h0 + 32, :])
            t1 = ropet.tile([128, TOKC], BF16, tag="ropet1")
            nc.vector.tensor_mul(t1, raw, cos_sl)
            rot2 = ropet.tile([128, TOKC], BF16, tag="ropet2")
            nc.vector.tensor_mul(rot2, rot, sin_sl)
            nc.vector.tensor_add(dst, t1, rot2)

        def proj_chunk(R):
            """Emits the x DMAs for chunk R immediately (prefetch) and
            returns (steps, qts): `steps` is a list of ~0.5-1us closures
            (2-4 matmuls or one rope each) the caller interleaves between
            attention t-steps to keep the PE army fed."""
            tsl = slice(R * TOKC, (R + 1) * TOKC)
            cos_sl = cos_sb[:, tsl]
            sin_sl = sin_sb[:, tsl]

            xq_t, xk_t, xv_t = [], [], []
            for k2 in range(KT // 2):
                t = xstream.tile([128, 2, TOKC], BF16, tag="xqs", bufs=9,
                                 name="xq_t")
                nc.sync.dma_start(out=t, in_=xq_r[:, k2, :, tsl])
                xq_t.append(t)
                t = xstream.tile([128, 2, TOKC], BF16, tag="xks", name="xk_t")
                nc.sync.dma_start(out=t, in_=xk_r[:, k2, :, tsl])
                xk_t.append(t)
                t = xstream.tile([128, 2, TOKC], BF16, tag="xvs", name="xv_t")
                nc.sync.dma_start(out=t, in_=xv_r[:, k2, :, tsl])
                xv_t.append(t)

            qts = [qtp.tile([128, TOKC], BF16, tag=f"qt{c}", name=f"qt{c}")
                   for c in range(QCOLS // 128)]
            steps = []

            # Q sweep 1 (cols 0,1), K chain, Q sweep 2 (cols 2,3), V chain:
            # the K/V chains cover the rope-evict latency of the Q sweeps.
            # PSUM chain tiles are allocated lazily inside the first step of
            # each chain so slot-rotation order matches the interleaved
            # execution order (eager allocation would make the attention
            # normalizes wait on late projection chains, or vice versa).
            def q_sweep(cs):
                box = {}

                def mms(k2):
                    if k2 == 0:
                        box['psq'] = [
                            ps_a.tile([128, TOKC], F32, tag="acc", name=f"psq{c}")
                            for c in cs]
                    for dt in range(2):
                        kt = 2 * k2 + dt
                        for i, c in enumerate(cs):
                            nc.tensor.matmul(
                                box['psq'][i],
                                wq_sb[:, kt, c * 128:(c + 1) * 128],
                                xq_t[k2][:, dt, :],
                                start=(kt == 0), stop=(kt == KT - 1))
                for k2 in range(KT // 2):
                    steps.append(lambda k2=k2: mms(k2))
                for i, c in enumerate(cs):
                    steps.append(lambda i=i, c=c:
                                 rope_evict(box['psq'][i], qts[c], cos_sl, sin_sl))

            q_sweep((0, 1))

            kbox = {}

            def k_mms(k2):
                if k2 == 0:
                    kbox['psk'] = ps_a.tile([128, TOKC], F32, tag="acc",
                                            name="psk")
                for dt in range(2):
                    kt = 2 * k2 + dt
                    nc.tensor.matmul(kbox['psk'], wk_sb[:, kt, :],
                                     xk_t[k2][:, dt, :],
                                     start=(kt == 0), stop=(kt == KT - 1))
            for k2 in range(0, KT // 2, 2):
                steps.append(lambda k2=k2: (k_mms(k2), k_mms(k2 + 1)))

            def k_evict():
                rope_evict(kbox['psk'], kT_A[:, tsl], cos_sl, sin_sl)
                nc.gpsimd.tensor_copy(kT_B[0:64, tsl], kT_A[64:128, tsl])
                nc.gpsimd.tensor_copy(kT_B[64:128, tsl], kT_A[0:64, tsl])
            steps.append(k_evict)

            q_sweep((2, 3))

            # V: feature-major projection chain, then PE transposes to
            # token-major v_sb blocks.
            vbox = {}

            def v_mms(k2):
                if k2 == 0:
                    vbox['psv'] = ps_a.tile([128, TOKC], F32, tag="acc",
                                            name="psv")
                for dt in range(2):
                    kt = 2 * k2 + dt
                    nc.tensor.matmul(vbox['psv'], wv_sb[:, kt, :],
                                     xv_t[k2][:, dt, :],
                                     start=(kt == 0), stop=(kt == KT - 1))
            for k2 in range(0, KT // 2, 2):
                steps.append(lambda k2=k2: (v_mms(k2), v_mms(k2 + 1)))

            def v_stage():
                vbox['vstage'] = ropet.tile([128, TOKC], BF16, tag="vstage")
                nc.vector.tensor_copy(vbox['vstage'], vbox['psv'])
            steps.append(v_stage)

            def v_trans(tt):
                vstage = vbox['vstage']
                for h in range(KVHL):
                    pst = ps_a.tile([128, HD], BF16, tag="acc", name="pst")
                    nc.tensor.transpose(
                        pst, vstage[64 * h:64 * h + 64, tt * 128:(tt + 1) * 128],
                        ident[64 * h:64 * h + 64, :])
                    nc.vector.tensor_copy(v_sb[:, R * 4 + tt, h, 0:HD], pst)
            for tt in range(TOKC // 128):
                steps.append(lambda tt=tt: v_trans(tt))

            return steps, qts

        def attention_steps(R, qts):
            """Builds the attention work for row chunk R as a list of
            ~1us closures (one per kv tile: 2 packed score MMs + fused exp +
            mask + 2 ctx MMs), with per-pair normalizes and the a2a DMAs as
            their own steps."""
            nkv = 4 * R + 4
            ctxt = [ctxp.tile([128, TOKC], BF16, tag=f"ctxt{f}", name=f"ctxt{f}")
                    for f in range(QCOLS // 128)]
            steps = []

            def norm_head(h, cacc_w):
                den = denp.tile([1, TOKC], BF16, tag="den")
                nc.vector.tensor_copy(den, cacc_w[HD:HD + 1, :])
                bc = ps_a.tile([64, TOKC], F32, tag="acc")
                nc.tensor.matmul(bc, ones1, den, start=True, stop=True)
                rec = denp.tile([64, TOKC], F32, tag="rec", bufs=2)
                nc.vector.reciprocal_approx_fast(rec, bc)
                nc.vector.tensor_mul(
                    ctxt[h // 2][64 * (h % 2):64 * (h % 2) + 64, :],
                    cacc_w[0:HD, :], rec)

            def t_step(heads, ktile, cacc, t):
                ha, hb = heads
                j = t - 4 * R
                trim = 128 * j if j >= 0 else 0
                ksl = slice(t * 128, (t + 1) * 128)
                # both heads' scores land in the two banks of ONE psum
                # tile: packed row groups (0-63 / 64-127), one Exp for
                # both; columns below the causal diagonal are skipped.
                sc = ps_s.tile([128, 2, TOKC], F32, tag="sc")
                nc.tensor.matmul(sc[:, 0, trim:], ktile[0:64, ksl],
                                 qts[ha // 2][0:64, trim:],
                                 start=True, stop=True)
                nc.tensor.matmul(sc[:, 1, trim:], ktile[64:128, ksl],
                                 qts[hb // 2][64:128, trim:],
                                 start=True, stop=True)
                pr = probs.tile([128, 2, TOKC], BF16, tag="pr")
                nc.scalar.activation(pr[:, :, trim:], sc[:, :, trim:],
                                     AF.Exp, scale=SCALE)
                if j >= 0:
                    nc.vector.tensor_mul(pr[:, :, trim:], pr[:, :, trim:],
                                         msk_sb[:, j, :, trim:])
                for w in range(2):
                    kv = heads[w] // 4
                    nc.tensor.matmul(cacc[w][:, trim:],
                                     v_sb[:, t, kv, :],
                                     pr[:, w, trim:],
                                     start=(t == 0), stop=(t == nkv - 1),
                                     skip_group_check=True)

            # pair layout: w=0 head is even (q at partitions 0-63), w=1 head
            # is odd (q at 64-127); kT_A/kT_B supply the matching kv heads.
            for heads, ktile in [((0, 5), kT_A), ((2, 7), kT_A),
                                 ((4, 1), kT_B), ((6, 3), kT_B)]:
                cacc = [ps_c.tile([HD + 1, TOKC], F32, tag="cacc", name=f"cacc{w}")
                        for w in range(2)]
                for t in range(nkv):
                    steps.append(
                        lambda heads=heads, ktile=ktile, cacc=cacc, t=t:
                        t_step(heads, ktile, cacc, t))
                for w in range(2):
                    steps.append(lambda h=heads[w], cw=cacc[w]: norm_head(h, cw))

            def a2a_dmas():
                # chunk R covers dest row blocks 2R, 2R+1 of this batch
                for f in range(QCOLS // 128):
                    for h2 in range(2):
                        nc.sync.dma_start(
                            out=a2a_in[2 * R + h2, f],
                            in_=ctxt[f][:, h2 * OUT_ROWS:(h2 + 1) * OUT_ROWS])
            steps.append(a2a_dmas)
            return steps

        # Software-pipelined emission: attention R only needs k/v chunks <= R
        # and q chunk R.  The next chunk's projection steps are spliced
        # evenly BETWEEN attention R's t-steps, so while the Scalar engine
        # grinds through the exp backlog the PE always has independent
        # projection matmuls queued right behind — no idle windows for the
        # HAM clock throttle to trigger on.  wo (8MB, needed only at the
        # end) is loaded once chunk 0's DMAs are in the queues.
        wo_sb = wpool.tile([128, KT, DIM], BF16, tag="wo")
        psteps, qts = proj_chunk(0)
        for st in psteps:
            st()
        for R in range(NTOK):
            asteps = attention_steps(R, qts)
            if R + 1 < NTOK:
                psteps, qts_next = proj_chunk(R + 1)
            else:
                psteps, qts_next = [], None
            if R == 0:
                for kt in range(KT):
                    nc.sync.dma_start(out=wo_sb[:, kt, :],
                                      in_=wo[kt * 128:(kt + 1) * 128, :])
            na, npx = len(asteps), len(psteps)
            j = 0
            for i, st in enumerate(asteps):
                st()
                while j * na < (i + 1) * npx:
                    psteps[j]()
                    j += 1
            while j < npx:
                psteps[j]()
                j += 1
            qts = qts_next

        # ---- 8-core AllToAll + local out projection for own 2x256 rows ----
        nc.gpsimd.collective_compute(
            "AllToAll", mybir.AluOpType.bypass, replica_groups=A2A_GROUP,
            ins=[a2a_in[:, :, :, :]], outs=[a2a_out[:, :, :, :]])

        # slot s = src core s (batch s//4, feature block s%4); ctx_full holds
        # the full 2048 features for this core's 256-row slice of each batch
        ctx_full = ctxf.tile([128, KT, B, OUT_ROWS], BF16, tag="ctxf")
        for bo in range(B):
            for s4 in range(4):
                nc.sync.dma_start(
                    out=ctx_full[:, s4 * 4:(s4 + 1) * 4, bo, :],
                    in_=a2a_out[4 * bo + s4].rearrange("f p r -> p f r"))

        for bo in range(B):
            for rt in range(OUT_ROWS // 128):
                rsl = slice(rt * 128, (rt + 1) * 128)
                for oc in range(DIM // 512):
                    pso = ps_a.tile([128, 512], F32, tag="acc")
                    for kt in range(KT):
                        nc.tensor.matmul(pso, ctx_full[:, kt, bo, rsl],
                                         wo_sb[:, kt, oc * 512:(oc + 1) * 512],
                                         start=(kt == 0), stop=(kt == KT - 1))
                    orow = orow_p.tile([128, 512], F32, tag="orow")
                    nc.vector.tensor_copy(orow, pso)
                    nc.sync.dma_start(
                        out=out_ext[bo, rsl, oc * 512:(oc + 1) * 512], in_=orow)

    nc.finalize()
    return nc


_NC_CACHE = None


def _get_nc():
    global _NC_CACHE
    if _NC_CACHE is None:
        _NC_CACHE = _build_nc()
    return _NC_CACHE


def _rope_tables():
    idx = np.arange(0, HD, 2, dtype=np.float64) / HD
    inv_freq = 1.0 / 10000.0 ** idx  # RoPE factor branch: adj == 1 here
    pos = np.arange(S, dtype=np.float64)
    freqs = np.einsum("i,j->ij", pos, inv_freq)
    emb = np.concatenate([freqs, freqs], axis=-1)  # [S, HD]
    cos = np.cos(emb).astype(np.float32)
    sin = np.sin(emb).astype(np.float32)
    d = np.arange(128) % HD
    cosT = np.ascontiguousarray(cos[:, d].T)  # [128, S]
    sgn = np.where(d < HD // 2, -1.0, 1.0).astype(np.float32)
    sinT = np.ascontiguousarray(sin[:, d].T * sgn[:, None])
    return cosT.astype(NPBF), sinT.astype(NPBF)


def _masks():
    p = np.arange(128)[:, None]
    r = np.arange(TOKC)[None, :]
    m = np.stack([(128 * j + p <= r) for j in range(4)], axis=1)
    m = np.repeat(m[:, :, None, :], 2, axis=2)
    return np.ascontiguousarray(m.astype(NPBF))  # [128, 4, 2, TOKC]


def kernel(query, key, value, w_q, b_q, w_k, b_k, w_v, b_v, w_o, b_o,
           _trace=False, **_unused):
    for b in (b_q, b_k, b_v):
        assert np.abs(np.asarray(b)).max() == 0.0, "nonzero qkv bias unsupported"

    cosT, sinT = _rope_tables()
    msk = _masks()
    xqT = [np.ascontiguousarray(np.asarray(query)[b].T).astype(NPBF) for b in range(B)]
    xkT = [np.ascontiguousarray(np.asarray(key)[b].T).astype(NPBF) for b in range(B)]
    xvT = [np.ascontiguousarray(np.asarray(value)[b].T).astype(NPBF) for b in range(B)]
    w_q, w_k, w_v, w_o = (np.asarray(a) for a in (w_q, w_k, w_v, w_o))
    wo_bf = np.ascontiguousarray(w_o).astype(NPBF)

    in_maps = []
    for c in range(NCORES):
        b, cp = divmod(c, 4)
        in_maps.append({
            "xq": xqT[b], "xk": xkT[b], "xv": xvT[b],
            "wq": np.ascontiguousarray(w_q[:, cp * QCOLS:(cp + 1) * QCOLS]).astype(NPBF),
            "wk": np.ascontiguousarray(w_k[:, cp * KCOLS:(cp + 1) * KCOLS]).astype(NPBF),
            "wv": np.ascontiguousarray(w_v[:, cp * KCOLS:(cp + 1) * KCOLS]).astype(NPBF),
            "wo": wo_bf,
            "cosT": cosT, "sinT": sinT, "msk": msk,
        })

    nc = _get_nc()
    res = run_bass_kernel_spmd(nc, in_maps, list(range(NCORES)), trace=_trace)
    out = np.empty((B, S, DIM), np.float32)
    for c in range(NCORES):
        out[:, c * OUT_ROWS:(c + 1) * OUT_ROWS, :] = res.results[c]["out"]
    out += np.asarray(b_o)[None, None, :]
    if _trace:
        return out, res
    return out


# revision 46
# speedup vs baseline: 1.0339x; 1.0339x over previous
"""Grouped-Query Attention (B=2, S=2048, DIM=2048, 32 Q heads / 8 KV heads,
HD=64, RoPE, causal) on 8 Trainium2 NeuronCores.

Sharding: hybrid batch x tensor parallel. Core c handles batch b=c//4 and
head-group cp=c%4 (2 KV heads, 8 Q heads). Wq/Wk/Wv are column-sharded.
Each core outputs rows [c*256,(c+1)*256) of BOTH batches: an 8-core bf16
AllToAll of the per-head context makes every (src,dst) shard meaningful and
each core then multiplies its fully-gathered 2048-feature context slice by
the full Wo locally — no reduction.

All matmuls use bf16 inputs with fp32 PSUM accumulation. Activations stay
transposed [feature, token] so matmul contractions have the contraction dim
on partitions:
  qT = Wq^T x^T (RoPE on partition dim), kT likewise (each kv head
  duplicated at partition offsets 0/64 so the score matmuls of a q-head
  pair land in disjoint PE row groups and run concurrently),
  scoresT[kv, row] = kT^T qT, two kv tiles paired per 2-bank PSUM tile so
  one Exp activation covers 1024 columns,
  probsT = exp(scale*scoresT) in bf16 (no max subtraction: |scores*scale|
  < ~8 for this input distribution; softmax is shift-invariant),
  v is projected feature-major then PE-transposed to token-major with a
  ones column -> partition 64 of the ctx accumulator is the softmax
  denominator for free; it is broadcast with a rank-1 matmul,
  reciprocal'd (fast approx) on 64 lanes, and multiplied in.
Every PSUM tile holds exactly one matmul accumulation group — interleaved
start/stop groups inside one bank clobber each other on hardware.
"""

import numpy as np
from contextlib import ExitStack

import sys

if "/opt/trn_rl_repo" not in sys.path:
    sys.path.insert(0, "/opt/trn_rl_repo")

import ml_dtypes
import concourse.bass as bass
import concourse.bacc as bacc
import concourse.tile as tile
from concourse import mybir
from concourse.bass_utils import run_bass_kernel_spmd
from concourse.masks import make_identity

F32 = mybir.dt.float32
BF16 = mybir.dt.bfloat16
AF = mybir.ActivationFunctionType
NPBF = ml_dtypes.bfloat16

B, S, DIM = 2, 2048, 2048
QH, KVH, HD = 32, 8, 64
SCALE = HD ** -0.5

NCORES = 8
A2A_GROUP = [list(range(NCORES))]
QHL = 8            # q heads per core
KVHL = 2           # kv heads per core
QCOLS = QHL * HD   # 512
KCOLS = KVHL * HD  # 128
TOKC = 512         # token chunk (matmul N / PSUM bank width in fp32)
NTOK = S // TOKC   # 4
KT = DIM // 128    # 16 contraction tiles for the projections
OUT_ROWS = S // NCORES  # 256 output rows per core per batch


def _build_nc():
    nc = bacc.Bacc(None, num_devices=NCORES)

    xq = nc.declare_dram_parameter("xq", [DIM, S], BF16, isOutput=False)
    xk = nc.declare_dram_parameter("xk", [DIM, S], BF16, isOutput=False)
    xv = nc.declare_dram_parameter("xv", [DIM, S], BF16, isOutput=False)
    wq = nc.declare_dram_parameter("wq", [DIM, QCOLS], BF16, isOutput=False)
    wk = nc.declare_dram_parameter("wk", [DIM, KCOLS], BF16, isOutput=False)
    wv = nc.declare_dram_parameter("wv", [DIM, KCOLS], BF16, isOutput=False)
    wo = nc.declare_dram_parameter("wo", [DIM, DIM], BF16, isOutput=False)
    cosT = nc.declare_dram_parameter("cosT", [128, S], BF16, isOutput=False)
    sinT = nc.declare_dram_parameter("sinT", [128, S], BF16, isOutput=False)
    # mask[p, j, w, r] = 1.0 if 128*j + p <= r else 0.0 (causal mask for the
    # 4 diagonal kv tiles of each 512-token row chunk; duplicated along w so
    # one multiply covers both heads of a packed score tile)
    msk = nc.declare_dram_parameter("msk", [128, 4, 2, TOKC], BF16, isOutput=False)
    out_ext = nc.declare_dram_parameter("out", [B, OUT_ROWS, DIM], F32, isOutput=True)

    # AllToAll buffers: [dest/src core, local feat tile, 128, 256 rows]
    a2a_in = nc.dram_tensor("a2a_in", [NCORES, 4, 128, OUT_ROWS], BF16)
    a2a_out = nc.dram_tensor("a2a_out", [NCORES, 4, 128, OUT_ROWS], BF16)

    with tile.TileContext(nc) as tc, ExitStack() as ctx:
        const = ctx.enter_context(tc.tile_pool(name="const", bufs=1))
        wpool = ctx.enter_context(tc.tile_pool(name="wpool", bufs=1))
        qkv = ctx.enter_context(tc.tile_pool(name="qkv", bufs=1))
        qtp = ctx.enter_context(tc.tile_pool(name="qtp", bufs=2))
        xstream = ctx.enter_context(tc.tile_pool(name="xstream", bufs=3))
        probs = ctx.enter_context(tc.tile_pool(name="probs", bufs=3))
        ropet = ctx.enter_context(tc.tile_pool(name="ropet", bufs=2))
        denp = ctx.enter_context(tc.tile_pool(name="denp", bufs=4))
        ctxp = ctx.enter_context(tc.tile_pool(name="ctxp", bufs=2))
        ctxf = ctx.enter_context(tc.tile_pool(name="ctxf", bufs=1))
        orow_p = ctx.enter_context(tc.tile_pool(name="orow", bufs=2))
        ps_a = ctx.enter_context(tc.tile_pool(name="ps_a", bufs=2, space="PSUM"))
        ps_s = ctx.enter_context(tc.tile_pool(name="ps_s", bufs=2, space="PSUM"))
        ps_c = ctx.enter_context(tc.tile_pool(name="ps_c", bufs=2, space="PSUM"))

        # ---- constants / weights resident in SBUF ----
        ones1 = const.tile([1, 64], BF16, tag="ones1")
        nc.vector.memset(ones1, 1.0)
        # identity duplicated in both partition halves for the v transposes
        ident = const.tile([128, 64], BF16, tag="ident")
        make_identity(nc, ident[0:64, :])
        make_identity(nc, ident[64:128, :])

        # wq + chunk-0 x tiles gate the first matmul — load them first
        wq_sb = wpool.tile([128, KT, QCOLS], BF16, tag="wq")
        for kt in range(KT):
            nc.sync.dma_start(out=wq_sb[:, kt, :],
                              in_=wq[kt * 128:(kt + 1) * 128, :])
        wk_sb = wpool.tile([128, KT, KCOLS], BF16, tag="wk")
        nc.sync.dma_start(out=wk_sb, in_=wk.rearrange("(kt p) c -> p kt c", p=128))
        wv_sb = wpool.tile([128, KT, KCOLS], BF16, tag="wv")
        nc.sync.dma_start(out=wv_sb, in_=wv.rearrange("(kt p) c -> p kt c", p=128))

        cos_sb = const.tile([128, S], BF16, tag="cos")
        sin_sb = const.tile([128, S], BF16, tag="sin")
        for R in range(NTOK):
            sl = slice(R * TOKC, (R + 1) * TOKC)
            nc.sync.dma_start(out=cos_sb[:, sl], in_=cosT[:, sl])
            nc.sync.dma_start(out=sin_sb[:, sl], in_=sinT[:, sl])
        msk_sb = const.tile([128, 4, 2, TOKC], BF16, tag="msk")
        nc.sync.dma_start(out=msk_sb, in_=msk[:, :, :, :])

        # ---- persistent activations ----
        # kT_A = natural rope layout [kv0 @ 0-63, kv1 @ 64-127];
        # kT_B = swapped [kv1 @ 0-63, kv0 @ 64-127].  Head pairs are chosen
        # so each score matmul pair reads one of these directly.
        kT_A = qkv.tile([128, S], BF16, tag="ktA", name="ktA")
        kT_B = qkv.tile([128, S], BF16, tag="ktB", name="ktB")
        # v token-major with a ones column: [tok, kv_tile_idx, kv_head, 65]
        v_sb = qkv.tile([128, S // 128, KVHL, HD + 1], BF16, tag="v")
        nc.vector.memset(v_sb[:, :, :, HD:HD + 1], 1.0)

        xq_r = xq.rearrange("(k2 dt p) c -> p k2 dt c", dt=2, p=128)
        xk_r = xk.rearrange("(k2 dt p) c -> p k2 dt c", dt=2, p=128)
        xv_r = xv.rearrange("(k2 dt p) c -> p k2 dt c", dt=2, p=128)

        def rope_evict(ps, dst, cos_sl, sin_sl):
            """ps: [128, TOKC] fp32 PSUM with fresh projection; dst: bf16
            SBUF tile/slice. dst = ps*cos + rotate_half(ps)*sin.  The psum
            is first cast to bf16 on the Scalar engine so all DVE ops run
            in 2x/4x 16-bit modes."""
            raw = ropet.tile([128, TOKC], BF16, tag="rope_raw")
            nc.scalar.activation(raw, ps, AF.Copy)
            rot = ropet.tile([128, TOKC], BF16, tag="rot")
            for h0 in (0, 64):
                nc.vector.tensor_copy(rot[h0:h0 + 32, :], raw[h0 + 32:h0 + 64, :])
                nc.vector.tensor_copy(rot[h0 + 32:h0 + 64, :], raw[h0:h0 + 32, :])
            t1 = ropet.tile([128, TOKC], BF16, tag="ropet1")
            nc.vector.tensor_mul(t1, raw, cos_sl)
            rot2 = ropet.tile([128, TOKC], BF16, tag="ropet2")
            nc.vector.tensor_mul(rot2, rot, sin_sl)
            nc.vector.tensor_add(dst, t1, rot2)

        def proj_chunk(R):
            """Emits the x DMAs for chunk R immediately (prefetch) and
            returns (steps, qts): `steps` is a list of ~0.5-1us closures
            (2-4 matmuls or one rope each) the caller interleaves between
            attention t-steps to keep the PE army fed."""
            tsl = slice(R * TOKC, (R + 1) * TOKC)
            cos_sl = cos_sb[:, tsl]
            sin_sl = sin_sb[:, tsl]

            xq_t, xk_t, xv_t = [], [], []
            for k2 in range(KT // 2):
                t = xstream.tile([128, 2, TOKC], BF16, tag="xqs", bufs=9,
                                 name="xq_t")
                nc.sync.dma_start(out=t, in_=xq_r[:, k2, :, tsl])
                xq_t.append(t)
                t = xstream.tile([128, 2, TOKC], BF16, tag="xks", name="xk_t")
                nc.sync.dma_start(out=t, in_=xk_r[:, k2, :, tsl])
                xk_t.append(t)
                t = xstream.tile([128, 2, TOKC], BF16, tag="xvs", name="xv_t")
                nc.sync.dma_start(out=t, in_=xv_r[:, k2, :, tsl])
                xv_t.append(t)

            qts = [qtp.tile([128, TOKC], BF16, tag=f"qt{c}", name=f"qt{c}")
                   for c in range(QCOLS // 128)]
            steps = []

            # Q sweep 1 (cols 0,1), K chain, Q sweep 2 (cols 2,3), V chain:
            # the K/V chains cover the rope-evict latency of the Q sweeps.
            # PSUM chain tiles are allocated lazily inside the first step of
            # each chain so slot-rotation order matches the interleaved
            # execution order (eager allocation would make the attention
            # normalizes wait on late projection chains, or vice versa).
            def q_sweep(cs):
                box = {}

                def mms(k2):
                    if k2 == 0:
                        box['psq'] = [
                            ps_a.tile([128, TOKC], F32, tag="acc", name=f"psq{c}")
                            for c in cs]
                    for dt in range(2):
                        kt = 2 * k2 + dt
                        for i, c in enumerate(cs):
                            nc.tensor.matmul(
                                box['psq'][i],
                                wq_sb[:, kt, c * 128:(c + 1) * 128],
                                xq_t[k2][:, dt, :],
                                start=(kt == 0), stop=(kt == KT - 1))
                for k2 in range(KT // 2):
                    steps.append(lambda k2=k2: mms(k2))
                for i, c in enumerate(cs):
                    steps.append(lambda i=i, c=c:
                                 rope_evict(box['psq'][i], qts[c], cos_sl, sin_sl))

            q_sweep((0, 1))

            kbox = {}

            def k_mms(k2):
                if k2 == 0:
                    kbox['psk'] = ps_a.tile([128, TOKC], F32, tag="acc",
                                            name="psk")
                for dt in range(2):
                    kt = 2 * k2 + dt
                    nc.tensor.matmul(kbox['psk'], wk_sb[:, kt, :],
                                     xk_t[k2][:, dt, :],
                                     start=(kt == 0), stop=(kt == KT - 1))
            for k2 in range(0, KT // 2, 2):
                steps.append(lambda k2=k2: (k_mms(k2), k_mms(k2 + 1)))

            def k_evict():
                rope_evict(kbox['psk'], kT_A[:, tsl], cos_sl, sin_sl)
                nc.gpsimd.tensor_copy(kT_B[0:64, tsl], kT_A[64:128, tsl])
                nc.gpsimd.tensor_copy(kT_B[64:128, tsl], kT_A[0:64, tsl])
            steps.append(k_evict)

            q_sweep((2, 3))

            # V: feature-major projection chain, then PE transposes to
            # token-major v_sb blocks.
            vbox = {}

            def v_mms(k2):
                if k2 == 0:
                    vbox['psv'] = ps_a.tile([128, TOKC], F32, tag="acc",
                                            name="psv")
                for dt in range(2):
                    kt = 2 * k2 + dt
                    nc.tensor.matmul(vbox['psv'], wv_sb[:, kt, :],
                                     xv_t[k2][:, dt, :],
                                     start=(kt == 0), stop=(kt == KT - 1))
            for k2 in range(0, KT // 2, 2):
                steps.append(lambda k2=k2: (v_mms(k2), v_mms(k2 + 1)))

            def v_stage():
                vbox['vstage'] = ropet.tile([128, TOKC], BF16, tag="vstage", name="vstage")
                nc.vector.tensor_copy(vbox['vstage'], vbox['psv'])
            steps.append(v_stage)

            def v_trans(tt):
                vstage = vbox['vstage']
                for h in range(KVHL):
                    pst = ps_a.tile([128, HD], BF16, tag="acc", name="pst")
                    nc.tensor.transpose(
                        pst, vstage[64 * h:64 * h + 64, tt * 128:(tt + 1) * 128],
                        ident[64 * h:64 * h + 64, :])
                    nc.vector.tensor_copy(v_sb[:, R * 4 + tt, h, 0:HD], pst)
            for tt in range(TOKC // 128):
                steps.append(lambda tt=tt: v_trans(tt))

            return steps, qts

        def attention_steps(R, qts):
            """Builds the attention work for row chunk R as a list of
            ~1us closures (one per kv tile: 2 packed score MMs + fused exp +
            mask + 2 ctx MMs), with per-pair normalizes and the a2a DMAs as
            their own steps."""
            nkv = 4 * R + 4
            ctxt = [ctxp.tile([128, TOKC], BF16, tag=f"ctxt{f}", name=f"ctxt{f}")
                    for f in range(QCOLS // 128)]
            steps = []

            def norm_head(h, cacc_w):
                den = denp.tile([1, TOKC], BF16, tag="den")
                nc.vector.tensor_copy(den, cacc_w[HD:HD + 1, :])
                bc = ps_a.tile([64, TOKC], F32, tag="acc")
                nc.tensor.matmul(bc, ones1, den, start=True, stop=True)
                rec = denp.tile([64, TOKC], F32, tag="rec", bufs=2)
                nc.vector.reciprocal_approx_fast(rec, bc)
                nc.vector.tensor_mul(
                    ctxt[h // 2][64 * (h % 2):64 * (h % 2) + 64, :],
                    cacc_w[0:HD, :], rec)

            def t_step(heads, ktile, cacc, t):
                ha, hb = heads
                j = t - 4 * R
                trim = 128 * j if j >= 0 else 0
                ksl = slice(t * 128, (t + 1) * 128)
                # both heads' scores land in the two banks of ONE psum
                # tile: packed row groups (0-63 / 64-127), one Exp for
                # both; columns below the causal diagonal are skipped.
                sc = ps_s.tile([128, 2, TOKC], F32, tag="sc")
                nc.tensor.matmul(sc[:, 0, trim:], ktile[0:64, ksl],
                                 qts[ha // 2][0:64, trim:],
                                 start=True, stop=True)
                nc.tensor.matmul(sc[:, 1, trim:], ktile[64:128, ksl],
                                 qts[hb // 2][64:128, trim:],
                                 start=True, stop=True)
                pr = probs.tile([128, 2, TOKC], BF16, tag="pr")
                nc.scalar.activation(pr[:, :, trim:], sc[:, :, trim:],
                                     AF.Exp, scale=SCALE)
                if j >= 0:
                    nc.vector.tensor_mul(pr[:, :, trim:], pr[:, :, trim:],
                                         msk_sb[:, j, :, trim:])
                for w in range(2):
                    kv = heads[w] // 4
                    nc.tensor.matmul(cacc[w][:, trim:],
                                     v_sb[:, t, kv, :],
                                     pr[:, w, trim:],
                                     start=(t == 0), stop=(t == nkv - 1),
                                     skip_group_check=True)

            # pair layout: w=0 head is even (q at partitions 0-63), w=1 head
            # is odd (q at 64-127); kT_A/kT_B supply the matching kv heads.
            for heads, ktile in [((0, 5), kT_A), ((2, 7), kT_A),
                                 ((4, 1), kT_B), ((6, 3), kT_B)]:
                cacc = [ps_c.tile([HD + 1, TOKC], F32, tag="cacc", name=f"cacc{w}")
                        for w in range(2)]
                for t in range(nkv):
                    steps.append(
                        lambda heads=heads, ktile=ktile, cacc=cacc, t=t:
                        t_step(heads, ktile, cacc, t))
                for w in range(2):
                    steps.append(lambda h=heads[w], cw=cacc[w]: norm_head(h, cw))

            def a2a_dmas():
                # chunk R covers dest row blocks 2R, 2R+1 of this batch
                for f in range(QCOLS // 128):
                    for h2 in range(2):
                        nc.sync.dma_start(
                            out=a2a_in[2 * R + h2, f],
                            in_=ctxt[f][:, h2 * OUT_ROWS:(h2 + 1) * OUT_ROWS])
            steps.append(a2a_dmas)
            return steps

        # Software-pipelined emission: attention R only needs k/v chunks <= R
        # and q chunk R.  The next chunk's projection steps are spliced
        # evenly BETWEEN attention R's t-steps, so while the Scalar engine
        # grinds through the exp backlog the PE always has independent
        # projection matmuls queued right behind — no idle windows for the
        # HAM clock throttle to trigger on.  wo (8MB, needed only at the
        # end) is loaded once chunk 0's DMAs are in the queues.
        wo_sb = wpool.tile([128, KT, DIM], BF16, tag="wo")
        psteps, qts = proj_chunk(0)
        for st in psteps:
            st()
        for R in range(NTOK):
            asteps = attention_steps(R, qts)
            if R + 1 < NTOK:
                psteps, qts_next = proj_chunk(R + 1)
            else:
                psteps, qts_next = [], None
            if R == 0:
                for kt in range(KT):
                    nc.sync.dma_start(out=wo_sb[:, kt, :],
                                      in_=wo[kt * 128:(kt + 1) * 128, :])
            na, npx = len(asteps), len(psteps)
            j = 0
            for i, st in enumerate(asteps):
                st()
                while j * na < (i + 1) * npx:
                    psteps[j]()
                    j += 1
            while j < npx:
                psteps[j]()
                j += 1
            qts = qts_next

        # ---- 8-core AllToAll + local out projection for own 2x256 rows ----
        nc.gpsimd.collective_compute(
            "AllToAll", mybir.AluOpType.bypass, replica_groups=A2A_GROUP,
            ins=[a2a_in[:, :, :, :]], outs=[a2a_out[:, :, :, :]])

        # slot s = src core s (batch s//4, feature block s%4); ctx_full holds
        # the full 2048 features for this core's 256-row slice of each batch
        ctx_full = ctxf.tile([128, KT, B, OUT_ROWS], BF16, tag="ctxf")
        for bo in range(B):
            for s4 in range(4):
                nc.sync.dma_start(
                    out=ctx_full[:, s4 * 4:(s4 + 1) * 4, bo, :],
                    in_=a2a_out[4 * bo + s4].rearrange("f p r -> p f r"))

        for bo in range(B):
            for rt in range(OUT_ROWS // 128):
                rsl = slice(rt * 128, (rt + 1) * 128)
                for oc in range(DIM // 512):
                    pso = ps_a.tile([128, 512], F32, tag="acc")
                    for kt in range(KT):
                        nc.tensor.matmul(pso, ctx_full[:, kt, bo, rsl],
                                         wo_sb[:, kt, oc * 512:(oc + 1) * 512],
                                         start=(kt == 0), stop=(kt == KT - 1))
                    orow = orow_p.tile([128, 512], F32, tag="orow")
                    nc.vector.tensor_copy(orow, pso)
                    nc.sync.dma_start(
                        out=out_ext[bo, rsl, oc * 512:(oc + 1) * 512], in_=orow)

    nc.finalize()
    return nc


_NC_CACHE = None


def _get_nc():
    global _NC_CACHE
    if _NC_CACHE is None:
        _NC_CACHE = _build_nc()
    return _NC_CACHE


def _rope_tables():
    idx = np.arange(0, HD, 2, dtype=np.float64) / HD
    inv_freq = 1.0 / 10000.0 ** idx  # RoPE factor branch: adj == 1 here
    pos = np.arange(S, dtype=np.float64)
    freqs = np.einsum("i,j->ij", pos, inv_freq)
    emb = np.concatenate([freqs, freqs], axis=-1)  # [S, HD]
    cos = np.cos(emb).astype(np.float32)
    sin = np.sin(emb).astype(np.float32)
    d = np.arange(128) % HD
    cosT = np.ascontiguousarray(cos[:, d].T)  # [128, S]
    sgn = np.where(d < HD // 2, -1.0, 1.0).astype(np.float32)
    sinT = np.ascontiguousarray(sin[:, d].T * sgn[:, None])
    return cosT.astype(NPBF), sinT.astype(NPBF)


def _masks():
    p = np.arange(128)[:, None]
    r = np.arange(TOKC)[None, :]
    m = np.stack([(128 * j + p <= r) for j in range(4)], axis=1)
    m = np.repeat(m[:, :, None, :], 2, axis=2)
    return np.ascontiguousarray(m.astype(NPBF))  # [128, 4, 2, TOKC]


def kernel(query, key, value, w_q, b_q, w_k, b_k, w_v, b_v, w_o, b_o,
           _trace=False, **_unused):
    for b in (b_q, b_k, b_v):
        assert np.abs(np.asarray(b)).max() == 0.0, "nonzero qkv bias unsupported"

    cosT, sinT = _rope_tables()
    msk = _masks()
    xqT = [np.ascontiguousarray(np.asarray(query)[b].T).astype(NPBF) for b in range(B)]
    xkT = [np.ascontiguousarray(np.asarray(key)[b].T).astype(NPBF) for b in range(B)]
    xvT = [np.ascontiguousarray(np.asarray(value)[b].T).astype(NPBF) for b in range(B)]
    w_q, w_k, w_v, w_o = (np.asarray(a) for a in (w_q, w_k, w_v, w_o))
    wo_bf = np.ascontiguousarray(w_o).astype(NPBF)

    in_maps = []
    for c in range(NCORES):
        b, cp = divmod(c, 4)
        in_maps.append({
            "xq": xqT[b], "xk": xkT[b], "xv": xvT[b],
            "wq": np.ascontiguousarray(w_q[:, cp * QCOLS:(cp + 1) * QCOLS]).astype(NPBF),
            "wk": np.ascontiguousarray(w_k[:, cp * KCOLS:(cp + 1) * KCOLS]).astype(NPBF),
            "wv": np.ascontiguousarray(w_v[:, cp * KCOLS:(cp + 1) * KCOLS]).astype(NPBF),
            "wo": wo_bf,
            "cosT": cosT, "sinT": sinT, "msk": msk,
        })

    nc = _get_nc()
    res = run_bass_kernel_spmd(nc, in_maps, list(range(NCORES)), trace=_trace)
    out = np.empty((B, S, DIM), np.float32)
    for c in range(NCORES):
        out[:, c * OUT_ROWS:(c + 1) * OUT_ROWS, :] = res.results[c]["out"]
    out += np.asarray(b_o)[None, None, :]
    if _trace:
        return out, res
    return out
